# revision 35
# baseline (speedup 1.0000x reference)
"""DifferentialAttention Trainium2 kernel.

Sharding: 8 cores = batch(4) x head-group(2). Each core handles 1 batch and
4 query heads (2 kv heads) for both attention maps (q1/q2), computes the
partial output projection over its heads' rows of Wp; host sums the two
head-group partials per batch and adds the bias.

All matmuls run in fp16 (1 cyc/row on PE); softmax numerators/denominators in
fp32 PSUM. The Chebyshev window (48x48 grid, window 12) is handled by
row-band restriction of each key tile's query span plus a host-precomputed
multiplicative mask strip (col window + row edge validity) applied to the
exp() output.
"""

import os

import numpy as np

import concourse.bass as bass
import concourse.mybir as mybir
import concourse.tile as tile
from concourse.vector_clock import ScopedClock
from concourse.bass_utils import run_bass_kernel_spmd

# ---------------------------------------------------------------- constants
B, N, C = 4, 2304, 512
H, HKV = 8, 4
D = 64            # head dim
SIDE = 48         # grid side (48*48 = 2304)
WIN = 12          # Chebyshev window
NT = N // 128     # 18 token tiles
QC = 768          # query chunk for O' accumulation (16 grid rows)
NQC = N // QC     # 6
SBW = int(os.environ.get("K_SBW", "2048"))  # psum batch width
SBH = SBW // 2    # per-map half
STRIPW = 28 * SIDE  # max mask strip width (1344)
F16 = mybir.dt.float16
F32 = mybir.dt.float32
F8 = mybir.dt.float8e4
WSCALE = 16.0     # fp8 weight pre-scale; 1/WSCALE folded into lamin

_FIXED_DRAIN = False


def _fix_tile_drain():
    """walrus CTRL lowering rejects >2 sync waits on the tail Drain; spread
    the global-clock waits over single-wait NOPs instead."""
    global _FIXED_DRAIN
    if _FIXED_DRAIN:
        return
    _FIXED_DRAIN = True

    MAXW = 1

    def _spread_excess_waits(nc):
        """walrus core_v3 codegen accepts at most 2 sync waits per
        instruction; move the excess onto NOPs inserted just before."""
        for f in nc.m.functions:
            for bb in f.blocks:
                insts = bb.instructions
                i = 0
                new = []
                for inst in insts:
                    si = inst.sync_info
                    if si is not None and len(si.on_wait) > MAXW:
                        waits = list(si.on_wait)
                        keep = waits[:MAXW]
                        rest = waits[MAXW:]
                        del si.on_wait[:]
                        for w in keep:
                            si.on_wait.append(w)
                        for k in range(0, len(rest), MAXW):
                            nop = mybir.InstNoOp(
                                name=f"I-{nc.next_id()}",
                                engine=inst.engine,
                                ins=[], outs=[],
                                sync_info=mybir.SyncInfo(
                                    on_wait=rest[k:k + MAXW], on_update=[]),
                            )
                            nc.register_instruction(nop, overwrite=True)
                            new.append(nop)
                    new.append(inst)
                    i += 1
                if len(new) != len(insts):
                    del insts[:]
                    for inst in new:
                        insts.append(inst)

    def _patched(self, tick_clock, wait_clock):
        drain_inst = self.nc.sync.drain()
        wait_clock.add_sem_waits(
            drain_inst.ins, ScopedClock({None: tick_clock.global_clock})
        )
        self.nc.all_engine_barrier()
        popped = self.nc._tile_sem_poison_stack.pop()
        assert popped is self._sem_poison
        self.nc.clear_and_free_semaphores(list(self.sems.allocated().values()))
        self.nc.all_engine_barrier()
        _spread_excess_waits(self.nc)

    tile.TileContext._drain_and_barrier = _patched


# ---------------------------------------------------------------- band math
def _kt_rows(kt):
    r0 = (128 * kt) // SIDE
    r1 = (128 * kt + 127) // SIDE
    return r0, r1


def _kt_band(kt):
    """Valid query token range [qlo, qhi) for key tile kt."""
    r0, r1 = _kt_rows(kt)
    qlo = SIDE * max(0, r0 - WIN)
    qhi = SIDE * (min(SIDE - 1, r1 + WIN) + 1)
    return qlo, qhi


def _segments_for(h_unused, qc):
    """(kt, q0, w) segments for query chunk qc (tokens [qc*QC, qc*QC+QC))."""
    c0, c1 = qc * QC, qc * QC + QC
    segs = []
    for kt in range(NT):
        qlo, qhi = _kt_band(kt)
        s0, s1 = max(qlo, c0), min(qhi, c1)
        if s0 < s1:
            segs.append((kt, s0, s1 - s0))
    return segs


def _batches_for(qc):
    """Greedy-pack segments so each batch fits SBH columns per map half."""
    segs = _segments_for(None, qc)
    batches, cur, acc = [], [], 0
    for seg in segs:
        w = seg[2]
        if cur and acc + w > SBH:
            batches.append(cur)
            cur, acc = [], 0
        cur.append(seg)
        acc += w
    if cur:
        batches.append(cur)
    return batches


# ---------------------------------------------------------------- build bass
def _build_nc():
    _fix_tile_drain()
    phases = os.environ.get("K_PHASES", "ABNC")
    notrans = os.environ.get("K_NOTRANS", "0") == "1"
    nc = bass.Bass()

    # each core ships only its half of x^T; pair AllGather restores the full x
    xh = nc.dram_tensor("xh", [C, N // 2], F16, kind="ExternalInput")
    xhi = nc.dram_tensor("xhi", [C, N // 2], F16, kind="Internal")
    xg = nc.dram_tensor("xg", [2 * C, N // 2], F16, kind="Internal")
    # q/k weight columns in fp8 (prescaled x16; the QK L2-norm removes the
    # scale), v columns stay fp16
    wqk = nc.dram_tensor("wqk", [C, 768], F8, kind="ExternalInput")
    wv = nc.dram_tensor("wv", [C, 256], F16, kind="ExternalInput")
    wp = nc.dram_tensor("wp", [4 * D, C], F16, kind="ExternalInput")
    # packed constant tables, identical on every core: each core ships a
    # 1/8 column-slab; an 8-way AllGather restores the full pack.
    # layout: cols 0:576 = cos (32-wide rope), 576:1152 = sin,
    # 1152:1800 = mask factors (NT*28 rowok blocks + 3x48 colok tables)
    CW = NT * 32 * 2 + NT * 28 + 144  # 1800
    csh = nc.dram_tensor("csh", [128, CW // 8], F16, kind="ExternalInput")
    cshi = nc.dram_tensor("cshi", [128, CW // 8], F16, kind="Internal")
    csg = nc.dram_tensor("csg", [8 * 128, CW // 8], F16, kind="Internal")
    lamin = nc.dram_tensor("lamin", [128, 8], F32, kind="ExternalInput")
    dsc = nc.dram_tensor("dsc", [8, N], F16, kind="Internal")
    # full per-core partial projection; pair ReduceScatter leaves each core
    # with the summed half it returns to the host
    poin = nc.dram_tensor("poin", [N, C], F16, kind="Internal")
    rso = nc.dram_tensor("rso", [N // 2, C], F16, kind="Internal")
    out = nc.dram_tensor("out", [N // 2, C], F16, kind="ExternalOutput")

    with tile.TileContext(nc) as tc:
        with tc.tile_pool(name="persist", bufs=1) as P:
            # resident SBUF tensors
            xT_sb = [P.tile([128, N], F16, tag=f"xt{k}", name=f"xt{k}") for k in range(4)]
            wqk_sb = [P.tile([128, 768], F8, tag=f"wq{k}", name=f"wq{k}") for k in range(4)]
            wv_sb = [P.tile([128, 256], F16, tag=f"wv{k}", name=f"wv{k}") for k in range(4)]
            wp_sb = [P.tile([128, C], F16, tag=f"wp{k}", name=f"wp{k}") for k in range(2)]
            cos_sb = P.tile([128, NT * 32], F16, tag="cos", name="cos")
            sin_sb = P.tile([128, NT * 32], F16, tag="sin", name="sin")
            mf_sb = P.tile([128, NT * 28 + 144], F16, tag="mf", name="mf")
            ms_sb = P.tile([128, NT * STRIPW], F16, tag="ms", name="ms")
            lam_sb = P.tile([128, 8], F32, tag="lam", name="lam")
            # transposed activations: QT[h] rows0:64=map1 head h, 64:128=map2
            QT = [P.tile([128, N], F16, tag=f"qt{h}", name=f"qt{h}") for h in range(4)]
            KT = [P.tile([128, N], F16, tag=f"kt{j}", name=f"kt{j}") for j in range(2)]
            # V (+ones col): [128, kt, (m,j), 65]
            VT = P.tile([128, NT, 4, D + 1], F16, tag="vt", name="vt")
            # output of attention, transposed: rows = head dims
            OT = [P.tile([128, N], F16, tag=f"ot{g}", name=f"ot{g}") for g in range(2)]
            # denominators per (h, m): staging
            OE1 = [P.tile([D + 1, N], F16, tag=f"oe1{h}", name=f"oe1{h}") for h in range(4)]
            OE2 = [P.tile([D + 1, N], F16, tag=f"oe2{h}", name=f"oe2{h}") for h in range(4)]

            # dram->dram stage then pair AllGather; xg rows 0:C = pair-rank 0
            # half (tokens 0:N/2), rows C:2C = rank 1 half
            nc.sync.dma_start(out=xhi[:, :], in_=xh[:, :])
            nc.gpsimd.collective_compute(
                "AllGather", mybir.AluOpType.bypass,
                replica_groups=[[0, 1], [2, 3], [4, 5], [6, 7]],
                ins=[xhi[:, :]], outs=[xg[:, :]])
            for k in range(4):
                nc.sync.dma_start(out=xT_sb[k][:, 0:N // 2],
                                  in_=xg[128 * k:128 * (k + 1), :])
                nc.sync.dma_start(out=xT_sb[k][:, N // 2:N],
                                  in_=xg[C + 128 * k:C + 128 * (k + 1), :])
                nc.sync.dma_start(out=wqk_sb[k][:], in_=wqk[128 * k:128 * (k + 1), :])
                nc.sync.dma_start(out=wv_sb[k][:], in_=wv[128 * k:128 * (k + 1), :])
            for k in range(2):
                nc.sync.dma_start(out=wp_sb[k][:], in_=wp[128 * k:128 * (k + 1), :])
            # 8-way AllGather of the packed constant tables, then unpack the
            # column-slabs into their destination tiles
            nc.sync.dma_start(out=cshi[:, :], in_=csh[:, :])
            nc.gpsimd.collective_compute(
                "AllGather", mybir.AluOpType.bypass,
                replica_groups=[[0, 1, 2, 3, 4, 5, 6, 7]],
                ins=[cshi[:, :]], outs=[csg[:, :]])
            SLAB = CW // 8
            regions = [(0, 576, cos_sb), (576, 1152, sin_sb),
                       (1152, 1800, mf_sb)]
            for r in range(8):
                g0 = SLAB * r
                for lo, hi, dst in regions:
                    a, bnd = max(g0, lo), min(g0 + SLAB, hi)
                    if a < bnd:
                        nc.sync.dma_start(
                            out=dst[:, a - lo:bnd - lo],
                            in_=csg[128 * r:128 * (r + 1), a - g0:bnd - g0])
            nc.sync.dma_start(out=lam_sb[:], in_=lamin[:])
            nc.vector.memset(VT[:, :, :, D], 1.0)

            # expand mask strips on device: strip[p, r*48+c] =
            # rowok[p, kt*28+r] * colok[p, (kt%3)*48+c]
            msv = ms_sb[:]
            mfv = mf_sb[:]
            for kt in range(NT):
                qlo, qhi = _kt_band(kt)
                nb = (qhi - qlo) // SIDE
                dst = bass.AP(tensor=msv.tensor, offset=msv.offset + STRIPW * kt,
                              ap=[msv.ap[0], [SIDE, nb], [1, SIDE]])
                rok = bass.AP(tensor=mfv.tensor, offset=mfv.offset + 28 * kt,
                              ap=[mfv.ap[0], [1, nb], [0, SIDE]])
                cok = bass.AP(tensor=mfv.tensor,
                              offset=mfv.offset + NT * 28 + SIDE * (kt % 3),
                              ap=[mfv.ap[0], [0, nb], [1, SIDE]])
                nc.vector.tensor_mul(dst, cok, rok)

            # ---------------- Phase A: projections + rope + norm + transposes
            with tc.tile_pool(name="psA", bufs=2, space="PSUM") as psA, \
                 tc.tile_pool(name="sbA", bufs=2) as sbA:
                for m in range(NT):
                    pq = psA.tile([128, 1024], F32, tag="pq", name="pq")
                    for k in range(4):
                        for c0, c1 in ((0, 512), (512, 768)):
                            nc.tensor.matmul(
                                pq[:, c0:c1],
                                lhsT=xT_sb[k][:, 128 * m:128 * (m + 1)],
                                rhs=wqk_sb[k][:, c0:c1],
                                start=(k == 0), stop=(k == 3),
                            )
                        nc.tensor.matmul(
                            pq[:, 768:1024],
                            lhsT=xT_sb[k][:, 128 * m:128 * (m + 1)],
                            rhs=wv_sb[k][:],
                            start=(k == 0), stop=(k == 3),
                        )
                    # evict q/k to fp16 on ACT; squares for the norm too
                    qev = sbA.tile([128, 768], F16, tag="qev", name="qev")
                    nc.scalar.copy(qev[:], pq[:, 0:768])
                    sq = sbA.tile([128, 768], F32, tag="sq", name="sq")
                    nc.scalar.square(sq[:], pq[:, 0:768])
                    ssq = sbA.tile([128, 12], F32, tag="ssq", name="ssq")
                    nc.vector.tensor_reduce(
                        ssq[:], sq[:].rearrange("p (v d) -> p v d", d=D),
                        axis=mybir.AxisListType.X, op=mybir.AluOpType.add,
                    )
                    rt = sbA.tile([128, 12], F32, tag="rt", name="rt")
                    nc.scalar.sqrt(rt[:], ssq[:])
                    rsq = sbA.tile([128, 12], F32, tag="rsq", name="rsq")
                    nc.vector.reciprocal(rsq[:], rt[:])

                    # rope: t3_lo = q_lo*cos - q_hi*sin ; t3_hi = q_hi*cos + q_lo*sin
                    cosm = cos_sb[:, 32 * m:32 * (m + 1)]
                    sinm = sin_sb[:, 32 * m:32 * (m + 1)]
                    cosb = bass.AP(tensor=cosm.tensor, offset=cosm.offset,
                                   ap=[cosm.ap[0], [0, 12], [1, 32]])
                    sinb = bass.AP(tensor=sinm.tensor, offset=sinm.offset,
                                   ap=[sinm.ap[0], [0, 12], [1, 32]])
                    t1 = sbA.tile([128, 768], F16, tag="t1", name="t1")
                    t2 = sbA.tile([128, 768], F16, tag="t2", name="t2")
                    qv = qev[:].rearrange("p (v d) -> p v d", d=D)
                    t1v = t1[:].rearrange("p (v d) -> p v d", d=D)
                    t2v = t2[:].rearrange("p (v d) -> p v d", d=D)
                    nc.vector.tensor_mul(t1v[:, :, 0:32], qv[:, :, 0:32], cosb)
                    nc.vector.tensor_mul(t1v[:, :, 32:64], qv[:, :, 32:64], cosb)
                    nc.vector.tensor_mul(t2v[:, :, 0:32], qv[:, :, 32:64], sinb)
                    nc.vector.tensor_mul(t2v[:, :, 32:64], qv[:, :, 0:32], sinb)
                    t3 = sbA.tile([128, 768], F16, tag="t3", name="t3")
                    t3v = t3[:].rearrange("p (v d) -> p v d", d=D)
                    nc.vector.tensor_sub(t3v[:, :, 0:32], t1v[:, :, 0:32],
                                         t2v[:, :, 0:32])
                    nc.vector.tensor_add(t3v[:, :, 32:64], t1v[:, :, 32:64],
                                         t2v[:, :, 32:64])

                    # normalize all 12 vh, write natural buffer, transpose out
                    qn = sbA.tile([128, 768], F16, tag="qn", name="qn")
                    for v in range(12):
                        nc.vector.tensor_scalar_mul(
                            qn[:, D * v:D * (v + 1)],
                            t3[:, D * v:D * (v + 1)],
                            rsq[:, v:v + 1])
                    # col layout is pair-major: [q1_h|q2_h]*4, [k1_j|k2_j]*2
                    if not notrans:
                        for h in range(4):
                            nc.sync.dma_start_transpose(
                                QT[h][:, 128 * m:128 * (m + 1)],
                                qn[:, 128 * h:128 * (h + 1)])
                        for jj in range(2):
                            nc.sync.dma_start_transpose(
                                KT[jj][:, 128 * m:128 * (m + 1)],
                                qn[:, 512 + 128 * jj:512 + 128 * (jj + 1)])
                    # v eviction (cols 768:1024) -> VT[:, m, c, 0:64]
                    nc.scalar.copy(
                        VT[:, m, :, 0:D],
                        pq[:, 768:1024].rearrange("p (c d) -> p c d", d=D))

            # ---------------- Phase B: banded attention per (j, h, qc)
            with tc.tile_pool(name="psB", bufs=1, space="PSUM") as psB, \
                 tc.tile_pool(name="sbB", bufs=3) as sbB:
                for j in (range(2) if "B" in phases else ()):
                    for hh in (2 * j, 2 * j + 1):
                        for qc in range(NQC):
                            c0 = qc * QC
                            o1 = psB.tile([128, QC], F32, tag="o1", name="o1")
                            o2 = psB.tile([128, QC], F32, tag="o2", name="o2")
                            bank_started = [[False, False], [False, False]]
                            batches = _batches_for(qc)
                            for bi, segs in enumerate(batches):
                                sb = psB.tile(
                                    [128, SBW], F32, tag="sb", name="sb",
                                    bufs=int(os.environ.get("K_SBBUFS", "1")))
                                offs = []
                                off = 0
                                for (kt, q0, w) in segs:
                                    for mp, base in ((0, 0), (1, SBH)):
                                        # chunk MMs at 512-grid of sb
                                        a = base + off
                                        while a < base + off + w:
                                            b2 = min(a + 512 - (a % 512),
                                                     base + off + w)
                                            qa = q0 + (a - base - off)
                                            nc.tensor.matmul(
                                                sb[:, a:b2],
                                                lhsT=KT[j][64 * mp:64 * (mp + 1),
                                                           128 * kt:128 * (kt + 1)],
                                                rhs=QT[hh][64 * mp:64 * (mp + 1),
                                                           qa:qa + (b2 - a)],
                                                start=True, stop=True,
                                            )
                                            a = b2
                                    offs.append(off)
                                    off += w
                                # one exp over both written half-regions
                                u = sbB.tile([128, SBW], F16, tag="u", name="u")
                                sbv = sb[:]
                                uv = u[:]
                                sb3 = bass.AP(
                                    tensor=sbv.tensor, offset=sbv.offset,
                                    ap=[sbv.ap[0], [SBH, 2], [1, off]])
                                u3 = bass.AP(
                                    tensor=uv.tensor, offset=uv.offset,
                                    ap=[uv.ap[0], [SBH, 2], [1, off]])
                                nc.scalar.activation(
                                    u3, sb3,
                                    mybir.ActivationFunctionType.Exp,
                                    scale=1.0 / 8.0)
                                # mask strips + PV
                                for si_, (kt, q0, w) in enumerate(segs):
                                    qlo, _ = _kt_band(kt)
                                    mso = STRIPW * kt + (q0 - qlo)
                                    for base in (0, SBH):
                                        o = base + offs[si_]
                                        nc.vector.tensor_mul(
                                            u[:, o:o + w], u[:, o:o + w],
                                            ms_sb[:, mso:mso + w])
                                for si_, (kt, q0, w) in enumerate(segs):
                                    for mp, base, ot in ((0, 0, o1), (1, SBH, o2)):
                                        o = base + offs[si_]
                                        a = q0 - c0
                                        while a < q0 - c0 + w:
                                            b2 = min(a + 512 - (a % 512),
                                                     q0 - c0 + w)
                                            bank = a // 512
                                            st = not bank_started[mp][bank]
                                            bank_started[mp][bank] = True
                                            last = (bi == len(batches) - 1)
                                            nc.tensor.matmul(
                                                ot[0:D + 1, a:b2],
                                                lhsT=VT[:, kt, 2 * j + mp, :],
                                                rhs=u[:, o + (a - (q0 - c0)):o + (b2 - (q0 - c0))],
                                                start=st, stop=last,
                                                skip_group_check=True,
                                            )
                                            a = b2
                            # evict O' (incl. ones-row denominators) to fp16
                            nc.vector.tensor_copy(
                                OE1[hh][:, c0:c0 + QC], o1[0:D + 1, :])
                            nc.vector.tensor_copy(
                                OE2[hh][:, c0:c0 + QC], o2[0:D + 1, :])

            # ---------------- normalize + differential combine
            with tc.tile_pool(name="sbN", bufs=2) as sbN:
                for hh in (range(4) if "N" in phases else ()):
                    for mp, OE in ((0, OE1), (1, OE2)):
                        hm = 2 * hh + mp
                        dst = sbN.tile([128, NT], F16, tag="dst", name="dst")
                        nc.sync.dma_start(out=dst[:], in_=OE[hh][D:D + 1, :])
                        rin = sbN.tile([128, NT], F32, tag="rin", name="rin")
                        nc.vector.reciprocal(rin[:], dst[:])
                        nc.vector.tensor_scalar_mul(rin[:], rin[:],
                                                    lam_sb[:, hm:hm + 1])
                        r16 = sbN.tile([128, NT], F16, tag="r16", name="r16")
                        nc.vector.tensor_copy(r16[:], rin[:])
                        nc.sync.dma_start(out=dsc[hm:hm + 1, :], in_=r16[:])
                        bt = sbN.tile([D, N], F16, tag="bt", name="bt")
                        dr = dsc[hm:hm + 1, :]
                        nc.sync.dma_start(
                            out=bt[:],
                            in_=bass.AP(tensor=dr.tensor, offset=dr.offset,
                                        ap=[[0, D]] + dr.ap[1:]))
                        nc.vector.tensor_mul(OE[hh][0:D, :], OE[hh][0:D, :],
                                             bt[:])
                    g, r = divmod(hh, 2)
                    nc.vector.tensor_add(
                        OT[g][64 * r:64 * (r + 1), :],
                        OE1[hh][0:D, :], OE2[hh][0:D, :])

            # ---------------- Phase C: partial output projection
            with tc.tile_pool(name="psC", bufs=2, space="PSUM") as psC, \
                 tc.tile_pool(name="sbC", bufs=2) as sbC:
                for m in (range(NT) if "C" in phases else ()):
                    po = psC.tile([128, C], F32, tag="po", name="po")
                    for g in range(2):
                        nc.tensor.matmul(
                            po[:],
                            lhsT=OT[g][:, 128 * m:128 * (m + 1)],
                            rhs=wp_sb[g][:],
                            start=(g == 0), stop=(g == 1),
                        )
                    ev = sbC.tile([128, C], F16, tag="ev", name="ev")
                    nc.vector.tensor_copy(ev[:], po[:])
                    nc.sync.dma_start(out=poin[128 * m:128 * (m + 1), :], in_=ev[:])

            # ---------------- pair-sum the partials, return only our half
            with tc.tile_pool(name="sbR", bufs=2) as sbR:
                nc.gpsimd.collective_compute(
                    "ReduceScatter", mybir.AluOpType.add,
                    replica_groups=[[0, 1], [2, 3], [4, 5], [6, 7]],
                    ins=[poin[:, :]], outs=[rso[:, :]])
                for m in range(NT // 2):
                    rt = sbR.tile([128, C], F16, tag="rt", name="rt")
                    nc.sync.dma_start(out=rt[:], in_=rso[128 * m:128 * (m + 1), :])
                    nc.sync.dma_start(out=out[128 * m:128 * (m + 1), :], in_=rt[:])

    return nc


# ---------------------------------------------------------------- host prep
def _rope_tables():
    dq = D // 4
    inv = 1.0 / (10000.0 ** (np.arange(dq, dtype=np.float32) / dq))
    gh, gw = np.meshgrid(np.arange(SIDE, dtype=np.float32),
                         np.arange(SIDE, dtype=np.float32), indexing="ij")
    th = np.outer(gh.ravel(), inv)
    tw = np.outer(gw.ravel(), inv)
    theta = np.concatenate([th, tw], axis=1)
    theta = np.concatenate([theta, theta], axis=1)  # [N, 64]
    return np.cos(theta), np.sin(theta)


def _mask_factors():
    """[128, NT*28+144] fp16: rowok blocks per kt + 3 periodic col tables."""
    out = np.zeros((128, NT * 28 + 144), dtype=np.float16)
    p = np.arange(128)
    for kt in range(NT):
        qlo, qhi = _kt_band(kt)
        nb = (qhi - qlo) // SIDE
        kgh = (128 * kt + p)[:, None] // SIDE
        r = (qlo // SIDE + np.arange(nb))[None, :]
        out[:, 28 * kt:28 * kt + nb] = (np.abs(kgh - r) <= WIN)
    for t in range(3):
        kgw = ((32 * t + p) % SIDE)[:, None]
        c = np.arange(SIDE)[None, :]
        out[:, NT * 28 + SIDE * t:NT * 28 + SIDE * (t + 1)] = (
            np.abs(kgw - c) <= WIN)
    return out


_CACHE = {}


def _get_nc():
    if "nc" not in _CACHE:
        _CACHE["nc"] = _build_nc()
    return _CACHE["nc"]


def _get_runner():
    """Build the sharded PJRT callable once; reuse across kernel() calls."""
    if "runner" in _CACHE:
        return _CACHE["runner"]
    import jax
    import numpy as _np
    from jax.sharding import Mesh, PartitionSpec
    from jax.experimental.shard_map import shard_map
    import concourse.mybir as mb
    from concourse import bass2jax

    nc = _get_nc()
    bass2jax.install_neuronx_cc_hook()
    partition_name = (nc.partition_id_tensor.name
                      if nc.partition_id_tensor else None)
    in_names, out_names, out_avals, zero_outs = [], [], [], []
    in_shapes = []
    for alloc in nc.m.functions[0].allocations:
        if not isinstance(alloc, mb.MemoryLocationSet):
            continue
        name = alloc.memorylocations[0].name
        if alloc.kind == "ExternalInput":
            if name != partition_name:
                in_names.append(name)
                in_shapes.append((tuple(alloc.tensor_shape),
                                  mb.dt.np(alloc.dtype)))
        elif alloc.kind == "ExternalOutput":
            shape = tuple(alloc.tensor_shape)
            dtype = mb.dt.np(alloc.dtype)
            out_names.append(name)
            out_avals.append(jax.core.ShapedArray(shape, dtype))
            zero_outs.append(_np.zeros(shape, dtype))
    n_params = len(in_names)
    all_in = list(in_names) + list(out_names)
    if partition_name is not None:
        all_in.append(partition_name)
    donate = tuple(range(n_params, n_params + len(out_names)))

    def _body(*args):
        operands = list(args)
        if partition_name is not None:
            operands.append(bass2jax.partition_id_tensor())
        outs = bass2jax._bass_exec_p.bind(
            *operands,
            out_avals=tuple(out_avals),
            in_names=tuple(all_in),
            out_names=tuple(out_names),
            lowering_input_output_aliases=(),
            sim_require_finite=True,
            sim_require_nnan=True,
            nc=nc,
        )
        return tuple(outs)

    devices = jax.devices()[:8]
    mesh = Mesh(_np.asarray(devices), ("core",))
    in_specs = (PartitionSpec("core"),) * (n_params + len(out_names))
    out_specs = (PartitionSpec("core"),) * len(out_names)
    # our kernel writes every output element, so donated pre-zeroed output
    # buffers are not needed for correctness; skipping donation lets the
    # jitted callable be re-invoked with device-resident operands.
    sm = shard_map(_body, mesh=mesh, in_specs=in_specs, out_specs=out_specs,
                   check_rep=False)
    if os.environ.get("K_SLOWDISPATCH", "0") == "1":
        sharded = jax.jit(sm, keep_unused=True)
    else:
        # suppress bass_effect so calls take the C++ fast dispatch path
        arg_structs = [
            jax.ShapeDtypeStruct((8 * s[0], *s[1:]), dt)
            for (s, dt) in in_shapes
        ] + [
            jax.ShapeDtypeStruct((8 * a.shape[0], *a.shape[1:]), a.dtype)
            for a in out_avals
        ]
        sharded = bass2jax.fast_dispatch_compile(
            lambda: jax.jit(sm, keep_unused=True)
            .lower(*arg_structs).compile())

    def _concat(in_maps):
        concat_in = [
            np.concatenate([np.asarray(in_maps[c][name]) for c in range(8)],
                           axis=0)
            for name in in_names
        ]
        concat_zeros = [np.zeros((8 * z.shape[0], *z.shape[1:]), z.dtype)
                        for z in zero_outs]
        return concat_in + concat_zeros

    def run(in_maps):
        out_arrs = sharded(*_concat(in_maps))
        return [
            {name: np.asarray(out_arrs[i]).reshape(8, *out_avals[i].shape)[c]
             for i, name in enumerate(out_names)}
            for c in range(8)
        ]

    def bench(in_maps, iters=20):
        import time as _time
        import jax as _jax
        args = [_jax.device_put(a) for a in _concat(in_maps)]
        for a in args:
            a.block_until_ready()
        outs = sharded(*args)
        _jax.block_until_ready(outs)
        t0 = _time.perf_counter()
        for _ in range(iters):
            outs = sharded(*args)
        _jax.block_until_ready(outs)
        t1 = _time.perf_counter()
        return (t1 - t0) / iters

    _CACHE["runner"] = run
    _CACHE["bench"] = bench
    return run


def bench(in_maps, iters=20):
    _get_runner()
    return _CACHE["bench"](in_maps, iters)


def prepare_in_maps(x, Wq, Wk, Wv1, Wv2, lambda_p, Wp, bp):
    x = np.asarray(x, dtype=np.float32)
    Wq = np.asarray(Wq, dtype=np.float32)
    Wk = np.asarray(Wk, dtype=np.float32)
    Wv1 = np.asarray(Wv1, dtype=np.float32)
    Wv2 = np.asarray(Wv2, dtype=np.float32)
    lambda_p = np.asarray(lambda_p, dtype=np.float32)
    Wp = np.asarray(Wp, dtype=np.float32)
    bp = np.asarray(bp, dtype=np.float32)

    if "consts" not in _CACHE:
        cos, sin = _rope_tables()

        def fold(tab):  # [N, 32] -> [128, NT*32] per m-tile layout
            t = tab[:, 0:32].reshape(NT, 128, 32).transpose(1, 0, 2)
            return np.ascontiguousarray(t.reshape(128, NT * 32),
                                        dtype=np.float16)

        _CACHE["consts"] = np.concatenate(
            [fold(cos), fold(sin), _mask_factors()], axis=1)  # [128, 1800]
    cpack = _CACHE["consts"]
    lam = 1.0 / (1.0 + np.exp(-lambda_p.reshape(H)))  # sigmoid

    import ml_dtypes
    f8 = ml_dtypes.float8_e4m3
    in_maps = []
    core = 0
    for b in range(B):
        for hg in range(2):
            hs = 4 * hg          # first head of group
            cols = []
            for hl in range(4):          # q pairs: [q1_h | q2_h]
                g = 4 * hg + hl
                cols.append(Wq[:, 64 * g:64 * (g + 1)])
                cols.append(Wq[:, 512 + 64 * g:512 + 64 * (g + 1)])
            for jl in range(2):          # k pairs: [k1_j | k2_j]
                g = 2 * hg + jl
                cols.append(Wk[:, 64 * g:64 * (g + 1)])
                cols.append(Wk[:, 256 + 64 * g:256 + 64 * (g + 1)])
            wqk = np.concatenate(cols, axis=1)
            cols = []
            for jl in range(2):          # v pairs: [v1_j | v2_j]
                g = 2 * hg + jl
                cols.append(Wv1[:, 64 * g:64 * (g + 1)])
                cols.append(Wv2[:, 64 * g:64 * (g + 1)])
            wv = np.concatenate(cols, axis=1)
            # lamin rows: hm = 2*h + m ; m=0 -> +1, m=1 -> -lam_h
            lamin = np.ones((128, 8), dtype=np.float32)
            for hl in range(4):
                lamin[:, 2 * hl + 1] = -lam[hs + hl]
            SLAB = 1800 // 8
            in_maps.append({
                "xh": np.ascontiguousarray(
                    x[b].T[:, hg * (N // 2):(hg + 1) * (N // 2)],
                    dtype=np.float16),
                "wqk": (wqk * WSCALE).astype(f8),
                "wv": wv.astype(np.float16),
                "wp": np.ascontiguousarray(
                    Wp[256 * hg:256 * (hg + 1), :], dtype=np.float16),
                "csh": np.ascontiguousarray(
                    cpack[:, SLAB * core:SLAB * (core + 1)]),
                "lamin": lamin,
            })
            core += 1
    return in_maps


def kernel(x, Wq, Wk, Wv1, Wv2, lambda_p, Wp, bp):
    bp = np.asarray(bp, dtype=np.float32)
    in_maps = prepare_in_maps(x, Wq, Wk, Wv1, Wv2, lambda_p, Wp, bp)
    results = _get_runner()(in_maps)
    out = np.zeros((B, N, C), dtype=np.float32)
    for b in range(B):
        out[b, 0:N // 2] = results[2 * b]["out"].astype(np.float32) + bp
        out[b, N // 2:N] = results[2 * b + 1]["out"].astype(np.float32) + bp
    return out

